# revision 1
# baseline (speedup 1.0000x reference)
"""Trainium2 Bass kernel: batched multi-head attention.

  out = softmax(scale * (Q @ K^T)) @ V    per (batch, head)

Full shapes: Q/K/V [4, 16, 2048, 128] f32, scale [4, 16, 1, 1] f32.
Sharding: the 64 batch*head pairs are split across 8 NeuronCores
(8 heads per core, no cross-core communication).

Per-core kernel (per head):
  - load Q, K, V with s-on-partitions layout; PE-transpose Q and K into
    [d=128, S] layout (scale folded into Q^T during the PSUM->SBUF copy)
  - QK^T runs as a hi/lo fp16 split (3 fp16 matmuls accumulating in fp32
    PSUM: hi*hi + hi*lo + lo*hi), giving near-fp32 scores at 16-bit
    matmul throughput (native fp32 matmul is ~5-10x slower on TRN2)
  - per 128-row q-chunk: row-max on DVE, exp(S - m) on ScalarE with the
    row-sum accumulated for free (accum_out); P tiles PE-transposed
    (fp16) into a [t, s] P^T buffer in SBUF
  - per half-head (8 q-chunks): O^T[d, s] = sum_t V_t.T @ P^T_t in fp16
    with V stationary; PE-transpose O^T back to [s, d], scale rows by
    1/l, DMA out
"""

import numpy as np

import concourse.bass as bass
import concourse.mybir as mybir
import concourse.tile as tile
from concourse import bacc
from concourse.masks import make_identity

B, H, S, D = 4, 16, 2048, 128
N_CORES = 8
HEADS_PER_CORE = (B * H) // N_CORES  # 8

F32 = mybir.dt.float32
F16 = mybir.dt.float16
BF16 = mybir.dt.bfloat16
AX = mybir.AxisListType.X
EXP = mybir.ActivationFunctionType.Exp

# dtype of the probability matrix P (and V in the PV matmul)
P_DTYPE = F16
# QK matmul mode: "x2" = hi/lo fp16 3-matmul split (near-fp32 accuracy),
# "f16" = single fp16 matmul, "f32" = native fp32 matmul (slow)
QK_MODE = "x2"
# row-max: 0 = exact; 4 = stride-4 subsample + margin (requires bf16 P)
ROWMAX_SUB = 0
MARGIN = 25.0

TRACE = False
LAST_EXEC_NS = None


def _bcast_ap(ap, parts):
    """Broadcast a 1-element DRAM AP across `parts` partitions."""
    return bass.AP(
        tensor=ap.tensor,
        offset=ap.offset,
        ap=[[0, parts], [1, 1]],
    )


def build_attention_nc(
    n_heads=HEADS_PER_CORE,
    seq=S,
    p_dtype=None,
    qk_mode=None,
    rowmax_sub=None,
    repeat=1,
    ablate=frozenset(),
    bufs=None,
):
    import contextlib

    if p_dtype is None:
        p_dtype = P_DTYPE
    if qk_mode is None:
        qk_mode = QK_MODE
    if rowmax_sub is None:
        rowmax_sub = ROWMAX_SUB

    P = 128
    assert seq % P == 0
    bf = dict(raw=2, qkT=2, prow=2, psS=6, psT=2, osb=2, small=6)
    if bufs:
        bf.update(bufs)

    nc = bacc.Bacc("TRN2", target_bir_lowering=False)
    q_d = nc.declare_dram_parameter("q", [n_heads, seq, D], F32, isOutput=False)
    k_d = nc.declare_dram_parameter("k", [n_heads, seq, D], F32, isOutput=False)
    v_d = nc.declare_dram_parameter("v", [n_heads, seq, D], F32, isOutput=False)
    s_d = nc.declare_dram_parameter("scale", [n_heads, 1], F32, isOutput=False)
    o_d = nc.declare_dram_parameter("out", [n_heads, seq, D], F32, isOutput=True)

    with tile.TileContext(nc) as tc:
        with (
            tc.tile_pool(name="singles", bufs=1) as singles,
            tc.tile_pool(name="raw", bufs=bf["raw"]) as raw,
            tc.tile_pool(name="qkT", bufs=bf["qkT"]) as qkT,
            tc.tile_pool(name="prow", bufs=bf["prow"]) as prow,
            tc.tile_pool(name="ptb", bufs=1) as ptb,
            tc.tile_pool(name="stats", bufs=2) as stats,
            tc.tile_pool(name="small", bufs=bf["small"]) as small,
            tc.tile_pool(name="osb", bufs=bf["osb"]) as osb,
            tc.tile_pool(name="psS", bufs=bf["psS"], space="PSUM") as psS,
            tc.tile_pool(name="psT", bufs=bf["psT"], space="PSUM") as psT,
        ):
            pools = dict(
                singles=singles, raw=raw, qkT=qkT, prow=prow, ptb=ptb,
                stats=stats, small=small, osb=osb, psS=psS, psT=psT,
            )
            ident = singles.tile([P, P], F32, tag="ident")
            make_identity(nc, ident)
            if p_dtype != F32:
                ident_p = singles.tile([P, P], p_dtype, tag="identp")
                make_identity(nc, ident_p)
            else:
                ident_p = ident

            rep_ctx = (
                tc.For_i(0, repeat, 1) if repeat > 1 else contextlib.nullcontext()
            )
            with rep_ctx:
                _build_body(
                    nc, n_heads, seq, p_dtype, qk_mode, rowmax_sub,
                    q_d, k_d, v_d, s_d, o_d, pools, ident, ident_p, ablate,
                )

    nc.compile()
    return nc


def _build_body(
    nc, n_heads, seq, p_dtype, qk_mode, rowmax_sub,
    q_d, k_d, v_d, s_d, o_d, pools, ident, ident_p, ab,
):
    P = 128
    NQ = seq // P
    NT = seq // P
    NH = max(1, NQ // 2)
    half_s = NH * P
    n_halves = NQ // NH
    NSEG = seq // 512 if seq >= 512 else 1
    SEG = min(512, seq)
    cast_v = p_dtype != F32

    raw, qkT, prow, ptb = pools["raw"], pools["qkT"], pools["prow"], pools["ptb"]
    stats, small, osb = pools["stats"], pools["small"], pools["osb"]
    psS, psT = pools["psS"], pools["psT"]

    for h in range(n_heads):
        # ---- load inputs for this head ------------------------------
        scale_b = small.tile([P, 1], F32, tag="scaleb")
        nc.sync.dma_start(out=scale_b, in_=_bcast_ap(s_d[h], P))

        q_raw = raw.tile([P, NQ, D], F32, tag="qraw")
        k_raw = raw.tile([P, NT, D], F32, tag="kraw")
        v_sb = raw.tile([P, NT, D], F32, tag="vraw")
        if "noload" not in ab:
            nc.sync.dma_start(out=q_raw, in_=q_d[h].rearrange("(c p) d -> p c d", p=P))
            nc.sync.dma_start(out=k_raw, in_=k_d[h].rearrange("(c p) d -> p c d", p=P))
            nc.sync.dma_start(out=v_sb, in_=v_d[h].rearrange("(c p) d -> p c d", p=P))
        if cast_v and "noload" not in ab:
            v_mm = raw.tile([P, NT, D], p_dtype, tag="vcast")
            nc.gpsimd.tensor_copy(out=v_mm, in_=v_sb)
        else:
            v_mm = v_sb

        # ---- build Q^T (scaled) and K^T hi/lo  [d=128, seq] ---------
        # scale + fp16 hi/lo split happen in the raw [s, d] layout
        # (GpSimd + DVE), then fp16 tensors are block-transposed to
        # [d, s] via the DMA xbar (no PE involvement).
        if qk_mode == "f32":
            qTs = qkT.tile([P, seq], F32, tag="qTs")
            kTs = qkT.tile([P, seq], F32, tag="kTs")
            for g0 in ([] if "prep" in ab else range(0, NQ, 4)):
                gn = min(4, NQ - g0)
                tp = psT.tile([P, gn * P], F32, tag="t4")
                for j in range(gn):
                    nc.tensor.transpose(
                        tp[:, j * P : (j + 1) * P], q_raw[:, g0 + j, :], ident
                    )
                nc.vector.tensor_scalar_mul(
                    out=qTs[:, g0 * P : (g0 + gn) * P], in0=tp, scalar1=scale_b
                )
            for g0 in ([] if "prep" in ab else range(0, NT, 4)):
                gn = min(4, NT - g0)
                tp = psT.tile([P, gn * P], F32, tag="t4")
                for j in range(gn):
                    nc.tensor.transpose(
                        tp[:, j * P : (j + 1) * P], k_raw[:, g0 + j, :], ident
                    )
                nc.scalar.copy(out=kTs[:, g0 * P : (g0 + gn) * P], in_=tp)
        elif "prep" not in ab:
            need_qlo = qk_mode in ("x2", "x2b")
            need_klo = qk_mode == "x2"
            qTs = qkT.tile([P, seq], F32, tag="qTs")
            kTs = qkT.tile([P, seq], F32, tag="kTs")
            for g0 in range(0, NQ, 4):
                gn = min(4, NQ - g0)
                tp = psT.tile([P, gn * P], F32, tag="t4")
                for j in range(gn):
                    nc.tensor.transpose(
                        tp[:, j * P : (j + 1) * P], q_raw[:, g0 + j, :], ident
                    )
                nc.vector.tensor_scalar_mul(
                    out=qTs[:, g0 * P : (g0 + gn) * P], in0=tp, scalar1=scale_b
                )
            for g0 in range(0, NT, 4):
                gn = min(4, NT - g0)
                tp = psT.tile([P, gn * P], F32, tag="t4")
                for j in range(gn):
                    nc.tensor.transpose(
                        tp[:, j * P : (j + 1) * P], k_raw[:, g0 + j, :], ident
                    )
                nc.scalar.copy(out=kTs[:, g0 * P : (g0 + gn) * P], in_=tp)
            qT_hi = qkT.tile([P, seq], F16, tag="qhi")
            nc.gpsimd.tensor_copy(out=qT_hi, in_=qTs)
            kT_hi = qkT.tile([P, seq], F16, tag="khi")
            nc.gpsimd.tensor_copy(out=kT_hi, in_=kTs)
            if need_qlo:
                qT_lo = qkT.tile([P, seq], F16, tag="qlo")
                nc.vector.tensor_sub(out=qT_lo, in0=qTs, in1=qT_hi)
            if need_klo:
                kT_lo = qkT.tile([P, seq], F16, tag="klo")
                nc.vector.tensor_sub(out=kT_lo, in0=kTs, in1=kT_hi)

        rl = stats.tile([P, NQ], F32, tag="rl")

        for half in range(n_halves):
            qoff = half * NH
            pT = ptb.tile([P, NT, half_s], p_dtype, tag="pT")

            # ---- phase A/B: scores, softmax, P transpose ------------
            for qq in range(NH):
                qi = qoff + qq
                qs = slice(qi * P, (qi + 1) * P)

                sts = []
                NTILE = NSEG
                TW = SEG
                for jt in range(NTILE):
                    stt = psS.tile([P, TW], F32, tag="s1")
                    sts.append(stt)
                for j in range(NSEG):
                    st = sts[j]
                    a = j * SEG
                    if "qk" not in ab:
                        if qk_mode == "x2":
                            nc.tensor.matmul(
                                st, qT_hi[:, qs], kT_hi[:, a : a + SEG],
                                start=True, stop=False,
                            )
                            nc.tensor.matmul(
                                st, qT_hi[:, qs], kT_lo[:, a : a + SEG],
                                start=False, stop=False,
                            )
                            nc.tensor.matmul(
                                st, qT_lo[:, qs], kT_hi[:, a : a + SEG],
                                start=False, stop=True,
                            )
                        elif qk_mode == "x2b":
                            nc.tensor.matmul(
                                st, qT_hi[:, qs], kT_hi[:, a : a + SEG],
                                start=True, stop=False,
                            )
                            nc.tensor.matmul(
                                st, qT_lo[:, qs], kT_hi[:, a : a + SEG],
                                start=False, stop=True,
                            )
                        elif qk_mode == "f16":
                            nc.tensor.matmul(
                                st, qT_hi[:, qs], kT_hi[:, a : a + SEG]
                            )
                        else:
                            nc.tensor.matmul(
                                st, qTs[:, qs], kTs[:, a : a + SEG]
                            )

                m_parts = small.tile([P, NTILE], F32, tag="mparts")
                negm = small.tile([P, 1], F32, tag="negm")
                if "reduce" not in ab:
                    for j, stt in enumerate(sts):
                        if rowmax_sub > 1:
                            view = stt.rearrange(
                                "p (a b) -> p a b", b=rowmax_sub
                            )[:, :, 0]
                        else:
                            view = stt
                        nc.vector.reduce_max(m_parts[:, j : j + 1], view, axis=AX)
                    if rowmax_sub > 1:
                        negm0 = small.tile([P, 1], F32, tag="negm0")
                        nc.vector.reduce_max(negm0, m_parts, axis=AX, negate=True)
                        nc.scalar.add(out=negm, in_=negm0, add=-MARGIN)
                    else:
                        nc.vector.reduce_max(negm, m_parts, axis=AX, negate=True)

                p_row = prow.tile([P, seq], p_dtype, tag="prow")
                l_parts = small.tile([P, NTILE], F32, tag="lparts")
                if "exp" not in ab:
                    for j, stt in enumerate(sts):
                        nc.scalar.activation(
                            out=p_row[:, j * TW : (j + 1) * TW],
                            in_=stt,
                            func=EXP,
                            bias=negm,
                            accum_out=l_parts[:, j : j + 1],
                        )
                if "lsum" not in ab:
                    lsum = small.tile([P, 1], F32, tag="lsum")
                    nc.vector.reduce_sum(lsum, l_parts, axis=AX)
                    nc.vector.reciprocal(rl[:, qi : qi + 1], lsum)

                # transpose P row-block into pT (copies on DVE: fp16 2x mode)
                if "ptrans" not in ab:
                    GRP = 8 if (p_dtype != F32 and NT % 8 == 0) else 4
                    for gi, g0 in enumerate(range(0, NT, GRP)):
                        gn = min(GRP, NT - g0)
                        tp = psT.tile([P, gn * P], p_dtype, tag="t4")
                        for j in range(gn):
                            nc.tensor.transpose(
                                tp[:, j * P : (j + 1) * P],
                                p_row[:, (g0 + j) * P : (g0 + j + 1) * P],
                                ident_p,
                            )
                        if "pcopy" not in ab:
                            dst = pT[:, g0 : g0 + gn, qq * P : (qq + 1) * P]
                            srcv = tp.rearrange("p (a b) -> p a b", a=gn)
                            if gi % 2 == 0:
                                nc.vector.tensor_copy(out=dst, in_=srcv)
                            else:
                                nc.scalar.copy(out=dst, in_=srcv)

            # ---- phase C: O^T = sum_t V_t.T @ P^T_t -----------------
            # O^T segments live in the same 1-bank pool as score slices
            osegs = []
            for c in range(0, half_s, SEG):
                e = min(c + SEG, half_s)
                ot = psS.tile([P, e - c], F32, tag="s1", name=f"ot_{c}")
                osegs.append((ot, c, e))
            if "pv" not in ab:
                for tc_i in range(NT):
                    for ot, c, e in osegs:
                        nc.tensor.matmul(
                            ot,
                            v_mm[:, tc_i, :],
                            pT[:, tc_i, c:e],
                            start=(tc_i == 0),
                            stop=(tc_i == NT - 1),
                        )

            # ---- phase D: transpose back, normalize, store ----------
            oT_sb = osb.tile([P, half_s], p_dtype, tag="otsb")
            if "dtrans" not in ab:
                for ot, c, e in osegs:
                    nc.scalar.copy(out=oT_sb[:, c:e], in_=ot)

            o_sb = osb.tile([P, NH, D], F32, tag="osb")
            if "dtrans" in ab:
                nc.gpsimd.memset(o_sb, 0.0)
            for g0 in ([] if "dtrans" in ab else range(0, NH, 4)):
                gn = min(4, NH - g0)
                tp = psT.tile([P, gn * P], p_dtype, tag="t4")
                for j in range(gn):
                    nc.tensor.transpose(
                        tp[:, j * P : (j + 1) * P],
                        oT_sb[:, (g0 + j) * P : (g0 + j + 1) * P],
                        ident_p,
                    )
                for j in range(gn):
                    nc.vector.tensor_scalar_mul(
                        out=o_sb[:, g0 + j, :],
                        in0=tp[:, j * P : (j + 1) * P],
                        scalar1=rl[:, qoff + g0 + j : qoff + g0 + j + 1],
                    )
            nc.sync.dma_start(
                out=o_d[h].rearrange("(c p) d -> p c d", p=P)[
                    :, qoff : qoff + NH, :
                ],
                in_=o_sb,
            )


_NC_CACHE = {}


def _get_nc():
    key = (HEADS_PER_CORE, S, P_DTYPE, QK_MODE, ROWMAX_SUB)
    if key not in _NC_CACHE:
        _NC_CACHE[key] = build_attention_nc()
    return _NC_CACHE[key]


def kernel(query, key, value, scale_factor):
    global LAST_EXEC_NS
    from concourse.bass_utils import run_bass_kernel_spmd

    q = np.ascontiguousarray(np.asarray(query, dtype=np.float32).reshape(B * H, S, D))
    k = np.ascontiguousarray(np.asarray(key, dtype=np.float32).reshape(B * H, S, D))
    v = np.ascontiguousarray(np.asarray(value, dtype=np.float32).reshape(B * H, S, D))
    sc = np.ascontiguousarray(
        np.asarray(scale_factor, dtype=np.float32).reshape(B * H, 1)
    )

    nc = _get_nc()
    in_maps = []
    for c in range(N_CORES):
        sl = slice(c * HEADS_PER_CORE, (c + 1) * HEADS_PER_CORE)
        in_maps.append({"q": q[sl], "k": k[sl], "v": v[sl], "scale": sc[sl]})

    res = run_bass_kernel_spmd(nc, in_maps, list(range(N_CORES)), trace=TRACE)
    LAST_EXEC_NS = res.exec_time_ns
    outs = [np.asarray(res.results[c]["out"]) for c in range(N_CORES)]
    return np.concatenate(outs, axis=0).reshape(B, H, S, D).astype(np.float32)



# revision 6
# speedup vs baseline: 1.2435x; 1.2435x over previous
"""Trainium2 Bass kernel: batched multi-head attention.

  out = softmax(scale * (Q @ K^T)) @ V    per (batch, head)

Full shapes: Q/K/V [4, 16, 2048, 128] f32, scale [4, 16, 1, 1] f32.
Sharding: the 64 batch*head pairs are split across 8 NeuronCores
(8 heads per core, no cross-core communication).

Per-core kernel (per head):
  - load Q, K, V with s-on-partitions layout; PE-transpose Q and K into
    [d=128, S] layout (scale folded into Q^T during the PSUM->SBUF copy)
  - QK^T runs as a hi/lo fp16 split (3 fp16 matmuls accumulating in fp32
    PSUM: hi*hi + hi*lo + lo*hi), giving near-fp32 scores at 16-bit
    matmul throughput (native fp32 matmul is ~5-10x slower on TRN2)
  - per 128-row q-chunk: row-max on DVE, exp(S - m) on ScalarE with the
    row-sum accumulated for free (accum_out); P tiles PE-transposed
    (fp16) into a [t, s] P^T buffer in SBUF
  - per half-head (8 q-chunks): O^T[d, s] = sum_t V_t.T @ P^T_t in fp16
    with V stationary; PE-transpose O^T back to [s, d], scale rows by
    1/l, DMA out
"""

import numpy as np

import concourse.bass as bass
import concourse.mybir as mybir
import concourse.tile as tile
from concourse import bacc
from concourse.masks import make_identity

B, H, S, D = 4, 16, 2048, 128
N_CORES = 8
HEADS_PER_CORE = (B * H) // N_CORES  # 8

F32 = mybir.dt.float32
F32R = mybir.dt.float32r
F16 = mybir.dt.float16
BF16 = mybir.dt.bfloat16
AX = mybir.AxisListType.X
EXP = mybir.ActivationFunctionType.Exp

# dtype of the probability matrix P (and V in the PV matmul)
P_DTYPE = F16
# QK matmul mode: "f32r" = single TF32-like matmul at 16-bit PE rate
# (~2e-4 rel err), "x2" = hi/lo fp16 3-matmul split (near-fp32 accuracy),
# "f16" = single fp16 matmul, "f32" = native fp32 matmul (slow)
QK_MODE = "f32r"
# row-max: 0 = exact; 4 = stride-4 subsample + margin (requires bf16 P)
ROWMAX_SUB = 0
MARGIN = 25.0

TRACE = False
LAST_EXEC_NS = None


def _bcast_ap(ap, parts):
    """Broadcast a 1-element DRAM AP across `parts` partitions."""
    return bass.AP(
        tensor=ap.tensor,
        offset=ap.offset,
        ap=[[0, parts], [1, 1]],
    )


def build_attention_nc(
    n_heads=HEADS_PER_CORE,
    seq=S,
    p_dtype=None,
    qk_mode=None,
    rowmax_sub=None,
    repeat=1,
    ablate=frozenset(),
    bufs=None,
):
    import contextlib

    if p_dtype is None:
        p_dtype = P_DTYPE
    if qk_mode is None:
        qk_mode = QK_MODE
    if rowmax_sub is None:
        rowmax_sub = ROWMAX_SUB

    P = 128
    assert seq % P == 0
    bf = dict(raw=2, qkT=2, prow=2, psS=6, psT=2, osb=2, small=6)
    if bufs:
        bf.update(bufs)

    nc = bacc.Bacc("TRN2", target_bir_lowering=False)
    q_d = nc.declare_dram_parameter("q", [n_heads, seq, D], F32, isOutput=False)
    k_d = nc.declare_dram_parameter("k", [n_heads, seq, D], F32, isOutput=False)
    v_d = nc.declare_dram_parameter("v", [n_heads, seq, D], F32, isOutput=False)
    s_d = nc.declare_dram_parameter("scale", [n_heads, 1], F32, isOutput=False)
    o_d = nc.declare_dram_parameter("out", [n_heads, seq, D], F32, isOutput=True)

    with tile.TileContext(nc) as tc:
        with (
            tc.tile_pool(name="singles", bufs=1) as singles,
            tc.tile_pool(name="raw", bufs=bf["raw"]) as raw,
            tc.tile_pool(name="qkT", bufs=bf["qkT"]) as qkT,
            tc.tile_pool(name="prow", bufs=bf["prow"]) as prow,
            tc.tile_pool(name="ptb", bufs=1) as ptb,
            tc.tile_pool(name="stats", bufs=2) as stats,
            tc.tile_pool(name="small", bufs=bf["small"]) as small,
            tc.tile_pool(name="osb", bufs=bf["osb"]) as osb,
            tc.tile_pool(name="psS", bufs=bf["psS"], space="PSUM") as psS,
            tc.tile_pool(name="psT", bufs=bf["psT"], space="PSUM") as psT,
        ):
            pools = dict(
                singles=singles, raw=raw, qkT=qkT, prow=prow, ptb=ptb,
                stats=stats, small=small, osb=osb, psS=psS, psT=psT,
            )
            ident = singles.tile([P, P], F32, tag="ident")
            make_identity(nc, ident)
            if p_dtype != F32:
                ident_p = singles.tile([P, P], p_dtype, tag="identp")
                make_identity(nc, ident_p)
            else:
                ident_p = ident

            rep_ctx = (
                tc.For_i(0, repeat, 1) if repeat > 1 else contextlib.nullcontext()
            )
            with rep_ctx:
                _build_body(
                    nc, n_heads, seq, p_dtype, qk_mode, rowmax_sub,
                    q_d, k_d, v_d, s_d, o_d, pools, ident, ident_p, ablate,
                )

    nc.compile()
    return nc


def _build_body(
    nc, n_heads, seq, p_dtype, qk_mode, rowmax_sub,
    q_d, k_d, v_d, s_d, o_d, pools, ident, ident_p, ab,
):
    P = 128
    NQ = seq // P
    NT = seq // P
    NH = max(1, NQ // 2)
    half_s = NH * P
    n_halves = NQ // NH
    NSEG = seq // 512 if seq >= 512 else 1
    SEG = min(512, seq)
    cast_v = p_dtype != F32

    raw, qkT, prow, ptb = pools["raw"], pools["qkT"], pools["prow"], pools["ptb"]
    stats, small, osb = pools["stats"], pools["small"], pools["osb"]
    psS, psT = pools["psS"], pools["psT"]

    for h in range(n_heads):
        # ---- load inputs for this head ------------------------------
        scale_b = small.tile([P, 1], F32, tag="scaleb")
        nc.sync.dma_start(out=scale_b, in_=_bcast_ap(s_d[h], P))

        q_raw = raw.tile([P, NQ, D], F32, tag="qraw")
        k_raw = raw.tile([P, NT, D], F32, tag="kraw")
        v_sb = raw.tile([P, NT, D], F32, tag="vraw")
        if "noload" not in ab:
            nc.sync.dma_start(out=q_raw, in_=q_d[h].rearrange("(c p) d -> p c d", p=P))
            nc.sync.dma_start(out=k_raw, in_=k_d[h].rearrange("(c p) d -> p c d", p=P))
            nc.sync.dma_start(out=v_sb, in_=v_d[h].rearrange("(c p) d -> p c d", p=P))
        if cast_v and "noload" not in ab:
            v_mm = raw.tile([P, NT, D], p_dtype, tag="vcast")
            nc.gpsimd.tensor_copy(out=v_mm, in_=v_sb)
        else:
            v_mm = v_sb

        # ---- build Q^T (scaled) and K^T hi/lo  [d=128, seq] ---------
        # scale + fp16 hi/lo split happen in the raw [s, d] layout
        # (GpSimd + DVE), then fp16 tensors are block-transposed to
        # [d, s] via the DMA xbar (no PE involvement).
        if qk_mode == "f32":
            qTs = qkT.tile([P, seq], F32, tag="qTs")
            kTs = qkT.tile([P, seq], F32, tag="kTs")
            for g0 in ([] if "prep" in ab else range(0, NQ, 4)):
                gn = min(4, NQ - g0)
                tp = psT.tile([P, gn * P], F32, tag="t4")
                for j in range(gn):
                    nc.tensor.transpose(
                        tp[:, j * P : (j + 1) * P], q_raw[:, g0 + j, :], ident
                    )
                nc.vector.tensor_scalar_mul(
                    out=qTs[:, g0 * P : (g0 + gn) * P], in0=tp, scalar1=scale_b
                )
            for g0 in ([] if "prep" in ab else range(0, NT, 4)):
                gn = min(4, NT - g0)
                tp = psT.tile([P, gn * P], F32, tag="t4")
                for j in range(gn):
                    nc.tensor.transpose(
                        tp[:, j * P : (j + 1) * P], k_raw[:, g0 + j, :], ident
                    )
                nc.scalar.copy(out=kTs[:, g0 * P : (g0 + gn) * P], in_=tp)
        elif "prep" not in ab:
            need_qlo = qk_mode in ("x2", "x2b")
            need_klo = qk_mode == "x2"
            qkt_dt = F32R if qk_mode == "f32r" else F32
            qTs = qkT.tile([P, seq], qkt_dt, tag="qTs")
            kTs = qkT.tile([P, seq], qkt_dt, tag="kTs")
            for g0 in range(0, NQ, 4):
                gn = min(4, NQ - g0)
                tp = psT.tile([P, gn * P], F32, tag="t4")
                for j in range(gn):
                    nc.tensor.transpose(
                        tp[:, j * P : (j + 1) * P], q_raw[:, g0 + j, :], ident
                    )
                nc.vector.tensor_scalar_mul(
                    out=qTs[:, g0 * P : (g0 + gn) * P], in0=tp, scalar1=scale_b
                )
            for g0 in range(0, NT, 4):
                gn = min(4, NT - g0)
                tp = psT.tile([P, gn * P], F32, tag="t4")
                for j in range(gn):
                    nc.tensor.transpose(
                        tp[:, j * P : (j + 1) * P], k_raw[:, g0 + j, :], ident
                    )
                nc.scalar.copy(out=kTs[:, g0 * P : (g0 + gn) * P], in_=tp)
            if qk_mode != "f32r":
                qT_hi = qkT.tile([P, seq], F16, tag="qhi")
                nc.gpsimd.tensor_copy(out=qT_hi, in_=qTs)
                kT_hi = qkT.tile([P, seq], F16, tag="khi")
                nc.gpsimd.tensor_copy(out=kT_hi, in_=kTs)
            if need_qlo:
                qT_lo = qkT.tile([P, seq], F16, tag="qlo")
                nc.vector.tensor_sub(out=qT_lo, in0=qTs, in1=qT_hi)
            if need_klo:
                kT_lo = qkT.tile([P, seq], F16, tag="klo")
                nc.vector.tensor_sub(out=kT_lo, in0=kTs, in1=kT_hi)

        rl = stats.tile([P, NQ], F32, tag="rl")

        for half in range(n_halves):
            qoff = half * NH
            pT = ptb.tile([P, NT, half_s], p_dtype, tag="pT")

            # ---- phase A/B: scores, softmax, P transpose ------------
            for qq in range(NH):
                qi = qoff + qq
                qs = slice(qi * P, (qi + 1) * P)

                sts = []
                NTILE = NSEG
                TW = SEG
                for jt in range(NTILE):
                    stt = psS.tile([P, TW], F32, tag="s1")
                    sts.append(stt)
                for j in range(NSEG):
                    st = sts[j]
                    a = j * SEG
                    if "qk" not in ab:
                        if qk_mode == "f32r":
                            nc.tensor.matmul(
                                st, qTs[:, qs], kTs[:, a : a + SEG],
                                start=True, stop=True,
                            )
                        elif qk_mode == "x2":
                            nc.tensor.matmul(
                                st, qT_hi[:, qs], kT_hi[:, a : a + SEG],
                                start=True, stop=False,
                            )
                            nc.tensor.matmul(
                                st, qT_hi[:, qs], kT_lo[:, a : a + SEG],
                                start=False, stop=False,
                            )
                            nc.tensor.matmul(
                                st, qT_lo[:, qs], kT_hi[:, a : a + SEG],
                                start=False, stop=True,
                            )
                        elif qk_mode == "x2b":
                            nc.tensor.matmul(
                                st, qT_hi[:, qs], kT_hi[:, a : a + SEG],
                                start=True, stop=False,
                            )
                            nc.tensor.matmul(
                                st, qT_lo[:, qs], kT_hi[:, a : a + SEG],
                                start=False, stop=True,
                            )
                        elif qk_mode == "f16":
                            nc.tensor.matmul(
                                st, qT_hi[:, qs], kT_hi[:, a : a + SEG]
                            )
                        else:
                            nc.tensor.matmul(
                                st, qTs[:, qs], kTs[:, a : a + SEG]
                            )

                m_parts = small.tile([P, NTILE], F32, tag="mparts")
                negm = small.tile([P, 1], F32, tag="negm")
                if "reduce" not in ab:
                    for j, stt in enumerate(sts):
                        if rowmax_sub > 1:
                            view = stt.rearrange(
                                "p (a b) -> p a b", b=rowmax_sub
                            )[:, :, 0]
                        else:
                            view = stt
                        nc.vector.reduce_max(m_parts[:, j : j + 1], view, axis=AX)
                    if rowmax_sub > 1:
                        negm0 = small.tile([P, 1], F32, tag="negm0")
                        nc.vector.reduce_max(negm0, m_parts, axis=AX, negate=True)
                        nc.scalar.add(out=negm, in_=negm0, add=-MARGIN)
                    else:
                        nc.vector.reduce_max(negm, m_parts, axis=AX, negate=True)

                p_row = prow.tile([P, seq], p_dtype, tag="prow")
                l_parts = small.tile([P, NTILE], F32, tag="lparts")
                if "exp" not in ab:
                    for j, stt in enumerate(sts):
                        nc.scalar.activation(
                            out=p_row[:, j * TW : (j + 1) * TW],
                            in_=stt,
                            func=EXP,
                            bias=negm,
                            accum_out=l_parts[:, j : j + 1],
                        )
                if "lsum" not in ab:
                    lsum = small.tile([P, 1], F32, tag="lsum")
                    nc.vector.reduce_sum(lsum, l_parts, axis=AX)
                    nc.vector.reciprocal(rl[:, qi : qi + 1], lsum)

                # transpose P row-block into pT (copies on DVE: fp16 2x mode)
                if "ptrans" not in ab:
                    GRP = 8 if (p_dtype != F32 and NT % 8 == 0) else 4
                    for gi, g0 in enumerate(range(0, NT, GRP)):
                        gn = min(GRP, NT - g0)
                        tp = psT.tile([P, gn * P], p_dtype, tag="t4")
                        for j in range(gn):
                            nc.tensor.transpose(
                                tp[:, j * P : (j + 1) * P],
                                p_row[:, (g0 + j) * P : (g0 + j + 1) * P],
                                ident_p,
                            )
                        if "pcopy" not in ab:
                            dst = pT[:, g0 : g0 + gn, qq * P : (qq + 1) * P]
                            srcv = tp.rearrange("p (a b) -> p a b", a=gn)
                            if gi % 2 == 0:
                                nc.vector.tensor_copy(out=dst, in_=srcv)
                            else:
                                nc.scalar.copy(out=dst, in_=srcv)

            # ---- phase C: O^T = sum_t V_t.T @ P^T_t -----------------
            # O^T segments live in the same 1-bank pool as score slices
            osegs = []
            for c in range(0, half_s, SEG):
                e = min(c + SEG, half_s)
                ot = psS.tile([P, e - c], F32, tag="s1", name=f"ot_{c}")
                osegs.append((ot, c, e))
            if "pv" not in ab:
                for tc_i in range(NT):
                    for ot, c, e in osegs:
                        nc.tensor.matmul(
                            ot,
                            v_mm[:, tc_i, :],
                            pT[:, tc_i, c:e],
                            start=(tc_i == 0),
                            stop=(tc_i == NT - 1),
                        )

            # ---- phase D: transpose back, normalize, store ----------
            oT_sb = osb.tile([P, half_s], p_dtype, tag="otsb")
            if "dtrans" not in ab:
                for ot, c, e in osegs:
                    nc.scalar.copy(out=oT_sb[:, c:e], in_=ot)

            o_sb = osb.tile([P, NH, D], F32, tag="osb")
            if "dtrans" in ab:
                nc.gpsimd.memset(o_sb, 0.0)
            for g0 in ([] if "dtrans" in ab else range(0, NH, 4)):
                gn = min(4, NH - g0)
                tp = psT.tile([P, gn * P], p_dtype, tag="t4")
                for j in range(gn):
                    nc.tensor.transpose(
                        tp[:, j * P : (j + 1) * P],
                        oT_sb[:, (g0 + j) * P : (g0 + j + 1) * P],
                        ident_p,
                    )
                for j in range(gn):
                    nc.vector.tensor_scalar_mul(
                        out=o_sb[:, g0 + j, :],
                        in0=tp[:, j * P : (j + 1) * P],
                        scalar1=rl[:, qoff + g0 + j : qoff + g0 + j + 1],
                    )
            nc.sync.dma_start(
                out=o_d[h].rearrange("(c p) d -> p c d", p=P)[
                    :, qoff : qoff + NH, :
                ],
                in_=o_sb,
            )


_NC_CACHE = {}


def _get_nc():
    key = (HEADS_PER_CORE, S, P_DTYPE, QK_MODE, ROWMAX_SUB)
    if key not in _NC_CACHE:
        _NC_CACHE[key] = build_attention_nc()
    return _NC_CACHE[key]


def kernel(query, key, value, scale_factor):
    global LAST_EXEC_NS
    from concourse.bass_utils import run_bass_kernel_spmd

    q = np.ascontiguousarray(np.asarray(query, dtype=np.float32).reshape(B * H, S, D))
    k = np.ascontiguousarray(np.asarray(key, dtype=np.float32).reshape(B * H, S, D))
    v = np.ascontiguousarray(np.asarray(value, dtype=np.float32).reshape(B * H, S, D))
    sc = np.ascontiguousarray(
        np.asarray(scale_factor, dtype=np.float32).reshape(B * H, 1)
    )

    nc = _get_nc()
    in_maps = []
    for c in range(N_CORES):
        sl = slice(c * HEADS_PER_CORE, (c + 1) * HEADS_PER_CORE)
        in_maps.append({"q": q[sl], "k": k[sl], "v": v[sl], "scale": sc[sl]})

    res = run_bass_kernel_spmd(nc, in_maps, list(range(N_CORES)), trace=TRACE)
    LAST_EXEC_NS = res.exec_time_ns
    outs = [np.asarray(res.results[c]["out"]) for c in range(N_CORES)]
    return np.concatenate(outs, axis=0).reshape(B, H, S, D).astype(np.float32)



# revision 23
# speedup vs baseline: 4.1894x; 3.3691x over previous
"""Trainium2 Bass kernel: batched multi-head attention.

  out = softmax(scale * (Q @ K^T)) @ V    per (batch, head)

Full shapes: Q/K/V [4, 16, 2048, 128] f32, scale [4, 16, 1, 1] f32.
Sharding: the 64 batch*head pairs are split across 8 NeuronCores
(8 heads per core, no cross-core communication).

Per-core kernel (per head):
  - load Q, K, V with s-on-partitions layout; PE-transpose Q and K into
    [d=128, S] layout (scale folded into Q^T during the PSUM->SBUF copy)
  - QK^T runs as a hi/lo fp16 split (3 fp16 matmuls accumulating in fp32
    PSUM: hi*hi + hi*lo + lo*hi), giving near-fp32 scores at 16-bit
    matmul throughput (native fp32 matmul is ~5-10x slower on TRN2)
  - per 128-row q-chunk: row-max on DVE, exp(S - m) on ScalarE with the
    row-sum accumulated for free (accum_out); P tiles PE-transposed
    (fp16) into a [t, s] P^T buffer in SBUF
  - per half-head (8 q-chunks): O^T[d, s] = sum_t V_t.T @ P^T_t in fp16
    with V stationary; PE-transpose O^T back to [s, d], scale rows by
    1/l, DMA out
"""

import numpy as np

import concourse.bass as bass
import concourse.mybir as mybir
import concourse.tile as tile
from concourse import bacc
from concourse.masks import make_identity

B, H, S, D = 4, 16, 2048, 128
N_CORES = 8
HEADS_PER_CORE = (B * H) // N_CORES  # 8

F32 = mybir.dt.float32
F32R = mybir.dt.float32r
F16 = mybir.dt.float16
BF16 = mybir.dt.bfloat16
AX = mybir.AxisListType.X
EXP = mybir.ActivationFunctionType.Exp

# dtype of the probability matrix P (and V in the PV matmul)
P_DTYPE = BF16
# QK matmul mode: "f32r" = single TF32-like matmul at 16-bit PE rate
# (~2e-4 rel err), "x2" = hi/lo fp16 3-matmul split (near-fp32 accuracy),
# "f16" = single fp16 matmul, "f32" = native fp32 matmul (slow)
QK_MODE = "f32r"
# row-max: 0 = exact; 4 = stride-4 subsample + margin (requires bf16 P)
ROWMAX_SUB = 4
MARGIN = 5.0

TRACE = False
LAST_EXEC_NS = None


def _bcast_ap(ap, parts):
    """Broadcast a 1-element DRAM AP across `parts` partitions."""
    return bass.AP(
        tensor=ap.tensor,
        offset=ap.offset,
        ap=[[0, parts], [1, 1]],
    )


def build_attention_nc(
    n_heads=HEADS_PER_CORE,
    seq=S,
    p_dtype=None,
    qk_mode=None,
    rowmax_sub=None,
    repeat=1,
    ablate=frozenset(),
    bufs=None,
):
    import contextlib

    if p_dtype is None:
        p_dtype = P_DTYPE
    if qk_mode is None:
        qk_mode = QK_MODE
    if rowmax_sub is None:
        rowmax_sub = ROWMAX_SUB

    P = 128
    assert seq % P == 0
    bf = dict(raw=2, qkT=2, prow=2, ptb=2, psS=2, psO=2, psT=2, osb=2, small=12)
    if bufs:
        bf.update(bufs)

    nc = bacc.Bacc("TRN2", target_bir_lowering=False)
    q_d = nc.declare_dram_parameter("q", [n_heads, seq, D], F32, isOutput=False)
    k_d = nc.declare_dram_parameter("k", [n_heads, seq, D], F32, isOutput=False)
    v_d = nc.declare_dram_parameter("v", [n_heads, seq, D], F32, isOutput=False)
    s_d = nc.declare_dram_parameter("scale", [n_heads, 1], F32, isOutput=False)
    o_d = nc.declare_dram_parameter("out", [n_heads, seq, D], F32, isOutput=True)

    with tile.TileContext(nc) as tc:
        with (
            tc.tile_pool(name="singles", bufs=1) as singles,
            tc.tile_pool(name="raw", bufs=bf["raw"]) as raw,
            tc.tile_pool(name="qkT", bufs=bf["qkT"]) as qkT,
            tc.tile_pool(name="prow", bufs=bf["prow"]) as prow,
            tc.tile_pool(name="ptb", bufs=bf["ptb"]) as ptb,
            tc.tile_pool(name="stats", bufs=2) as stats,
            tc.tile_pool(name="small", bufs=bf["small"]) as small,
            tc.tile_pool(name="osb", bufs=bf["osb"]) as osb,
            tc.tile_pool(name="psS", bufs=bf["psS"], space="PSUM") as psS,
            tc.tile_pool(name="psO", bufs=bf["psO"], space="PSUM") as psO,
            tc.tile_pool(name="psT", bufs=bf["psT"], space="PSUM") as psT,
        ):
            pools = dict(
                singles=singles, raw=raw, qkT=qkT, prow=prow, ptb=ptb,
                stats=stats, small=small, osb=osb, psS=psS, psO=psO, psT=psT,
            )
            ident = singles.tile([P, P], F32, tag="ident")
            make_identity(nc, ident)
            if p_dtype != F32:
                ident_p = singles.tile([P, P], p_dtype, tag="identp")
                make_identity(nc, ident_p)
            else:
                ident_p = ident

            rep_ctx = (
                tc.For_i(0, repeat, 1) if repeat > 1 else contextlib.nullcontext()
            )
            with rep_ctx:
                _build_body(
                    nc, n_heads, seq, p_dtype, qk_mode, rowmax_sub,
                    q_d, k_d, v_d, s_d, o_d, pools, ident, ident_p, ablate,
                )

    nc.compile()
    return nc


def _build_body(
    nc, n_heads, seq, p_dtype, qk_mode, rowmax_sub,
    q_d, k_d, v_d, s_d, o_d, pools, ident, ident_p, ab,
):
    from concourse import bass_isa

    P = 128
    NQ = seq // P
    NT = seq // P
    NH = max(1, NQ // 2)
    half_s = NH * P
    SEG = min(512, seq)
    NTILE = 2
    TW = seq // NTILE
    cast_v = p_dtype != F32
    sub = rowmax_sub if rowmax_sub > 1 else 4
    PP_CHUNKS = [0, NQ // 4, NQ // 2, (3 * NQ) // 4][: max(1, NQ // 4)]

    raw, qkT, prow, ptb = pools["raw"], pools["qkT"], pools["prow"], pools["ptb"]
    stats, small, osb = pools["stats"], pools["small"], pools["osb"]
    psS, psO, psT = pools["psS"], pools["psO"], pools["psT"]

    heads = {}

    def prep(h):
        hd = {}
        scale_b = small.tile([P, 1], F32, tag="scaleb", name=f"scb{h}")
        nc.sync.dma_start(out=scale_b, in_=_bcast_ap(s_d[h], P))

        q_raw = raw.tile([P, NQ, D], F32, tag="qraw", name=f"qr{h}")
        k_raw = raw.tile([P, NT, D], F32, tag="kraw", name=f"kr{h}")
        v_sb = raw.tile([P, NT, D], F32, tag="vraw", name=f"vr{h}")
        if "noload" not in ab:
            nc.sync.dma_start(out=q_raw, in_=q_d[h].rearrange("(c p) d -> p c d", p=P))
            nc.sync.dma_start(out=k_raw, in_=k_d[h].rearrange("(c p) d -> p c d", p=P))
            nc.sync.dma_start(out=v_sb, in_=v_d[h].rearrange("(c p) d -> p c d", p=P))
        if cast_v and "noload" not in ab:
            v_mm = raw.tile([P, NT, D], p_dtype, tag="vcast", name=f"vc{h}")
            nc.gpsimd.tensor_copy(out=v_mm, in_=v_sb)
        else:
            v_mm = v_sb
        hd["v_mm"] = v_mm

        # Q^T (scaled) and K^T in fp32r: PE transpose (fp32), rounded to
        # f32r during the PSUM->SBUF copy (DVE for Q + scale, Pool for K)
        qkt_dt = F32R if qk_mode == "f32r" else F32
        qTs = qkT.tile([P, seq], qkt_dt, tag="qTs", name=f"qTs{h}")
        kTs = qkT.tile([P, seq], qkt_dt, tag="kTs", name=f"kTs{h}")
        for g0 in ([] if "prep" in ab else range(0, NQ, 4)):
            gn = min(4, NQ - g0)
            tp = psT.tile([P, gn * P], F32, tag="t4", name="tp")
            for j in range(gn):
                nc.tensor.transpose(
                    tp[:, j * P : (j + 1) * P], q_raw[:, g0 + j, :], ident
                )
            nc.vector.tensor_scalar_mul(
                out=qTs[:, g0 * P : (g0 + gn) * P], in0=tp, scalar1=scale_b
            )
        for g0 in ([] if "prep" in ab else range(0, NT, 4)):
            gn = min(4, NT - g0)
            tp = psT.tile([P, gn * P], F32, tag="t4", name="tp")
            for j in range(gn):
                nc.tensor.transpose(
                    tp[:, j * P : (j + 1) * P], k_raw[:, g0 + j, :], ident
                )
            nc.vector.tensor_copy(out=kTs[:, g0 * P : (g0 + gn) * P], in_=tp)
        hd["qTs"], hd["kTs"] = qTs, kTs
        if qk_mode not in ("f32r", "f32"):
            qT_hi = qkT.tile([P, seq], F16, tag="qhi", name=f"qh{h}")
            nc.gpsimd.tensor_copy(out=qT_hi, in_=qTs)
            kT_hi = qkT.tile([P, seq], F16, tag="khi", name=f"kh{h}")
            nc.gpsimd.tensor_copy(out=kT_hi, in_=kTs)
            hd["qT_hi"], hd["kT_hi"] = qT_hi, kT_hi
            if qk_mode in ("x2", "x2b"):
                qT_lo = qkT.tile([P, seq], F16, tag="qlo", name=f"ql{h}")
                nc.vector.tensor_sub(out=qT_lo, in0=qTs, in1=qT_hi)
                hd["qT_lo"] = qT_lo
            if qk_mode == "x2":
                kT_lo = qkT.tile([P, seq], F16, tag="klo", name=f"kl{h}")
                nc.vector.tensor_sub(out=kT_lo, in0=kTs, in1=kT_hi)
                hd["kT_lo"] = kT_lo

        # ---- per-head max prepass: subsampled scores -> scalar bound.
        # exp uses bias = -(M + MARGIN); bf16 P absorbs the dynamic range.
        negm_h = stats.tile([P, 1], F32, tag="negmh", name=f"nm{h}")
        if "reduce" not in ab:
            kview = kTs.rearrange("p (a b) -> p a b", b=sub)[:, :, 0]
            n_pp = len(PP_CHUNKS)
            m_pp = small.tile([P, n_pp], F32, tag="mpp", name=f"mpp{h}")
            done = 0
            while done < n_pp:
                take = min(2, n_pp - done)
                st = psS.tile([P, TW], F32, tag="s1", name=f"pp{h}_{done}")
                for u in range(take):
                    qi = PP_CHUNKS[done + u]
                    qs = slice(qi * P, (qi + 1) * P)
                    lhs = qTs if qk_mode in ("f32r", "f32") else hd["qT_hi"]
                    kv = kview if qk_mode in ("f32r", "f32") else (
                        hd["kT_hi"].rearrange("p (a b) -> p a b", b=sub)[:, :, 0]
                    )
                    nc.tensor.matmul(
                        st[:, u * (TW // 2) : u * (TW // 2) + seq // sub],
                        lhs[:, qs],
                        kv,
                        start=True,
                        stop=True,
                    )
                    nc.vector.reduce_max(
                        m_pp[:, done + u : done + u + 1],
                        st[:, u * (TW // 2) : u * (TW // 2) + seq // sub],
                        axis=AX,
                    )
                done += take
            mrow = small.tile([P, 1], F32, tag="mrow", name=f"mrow{h}")
            nc.vector.reduce_max(mrow, m_pp, axis=AX)
            mall = stats.tile([P, 1], F32, tag="mall", name=f"mall{h}")
            nc.gpsimd.partition_all_reduce(
                mall, mrow, channels=P, reduce_op=bass_isa.ReduceOp.max
            )
            nc.vector.tensor_scalar(
                out=negm_h, in0=mall, scalar1=-1.0, scalar2=-MARGIN,
                op0=mybir.AluOpType.mult, op1=mybir.AluOpType.add,
            )
        else:
            nc.vector.memset(negm_h, 0.0)
        hd["negm"] = negm_h

        hd["rl"] = stats.tile([P, NQ], F32, tag="rl", name=f"rl{h}")
        hd["rlraw"] = stats.tile([P, NQ], F32, tag="rlraw", name=f"rlw{h}")
        heads[h] = hd

    def emit_qk(h, qi):
        hd = heads[h]
        qs = slice(qi * P, (qi + 1) * P)
        sts = [
            psS.tile([P, TW], F32, tag="s1", name=f"s{h}_{qi}_{j}")
            for j in range(NTILE)
        ]
        if "qk" in ab:
            return sts
        for j in range(NTILE):
            st = sts[j]
            for s0 in range(0, TW, SEG):
                a = j * TW + s0
                dst = st[:, s0 : s0 + SEG]
                if qk_mode in ("f32r", "f32"):
                    nc.tensor.matmul(
                        dst, hd["qTs"][:, qs], hd["kTs"][:, a : a + SEG],
                        start=True, stop=True,
                    )
                elif qk_mode == "x2":
                    nc.tensor.matmul(
                        dst, hd["qT_hi"][:, qs], hd["kT_hi"][:, a : a + SEG],
                        start=True, stop=False,
                    )
                    nc.tensor.matmul(
                        dst, hd["qT_hi"][:, qs], hd["kT_lo"][:, a : a + SEG],
                        start=False, stop=False,
                    )
                    nc.tensor.matmul(
                        dst, hd["qT_lo"][:, qs], hd["kT_hi"][:, a : a + SEG],
                        start=False, stop=True,
                    )
                elif qk_mode == "x2b":
                    nc.tensor.matmul(
                        dst, hd["qT_hi"][:, qs], hd["kT_hi"][:, a : a + SEG],
                        start=True, stop=False,
                    )
                    nc.tensor.matmul(
                        dst, hd["qT_lo"][:, qs], hd["kT_hi"][:, a : a + SEG],
                        start=False, stop=True,
                    )
                else:
                    nc.tensor.matmul(
                        dst, hd["qT_hi"][:, qs], hd["kT_hi"][:, a : a + SEG]
                    )
        return sts

    def emit_pv(pend, tlist):
        if "pv" in ab:
            return
        for tc_i in tlist:
            for ot, c, e in pend["osegs"]:
                nc.tensor.matmul(
                    ot,
                    pend["v_mm"][:, tc_i, :],
                    pend["pT"][:, tc_i, c:e],
                    start=(tc_i == 0),
                    stop=(tc_i == NT - 1),
                )

    def emit_phase_d(pend):
        h, qoff, rl = pend["h"], pend["qoff"], pend["rl"]
        oT_sb = osb.tile([P, half_s], p_dtype, tag="otsb", name="oTsb")
        if "dtrans" not in ab:
            for ot, c, e in pend["osegs"]:
                nc.vector.tensor_copy(out=oT_sb[:, c:e], in_=ot)
        o_sb = osb.tile([P, NH, D], F32, tag="osb", name="osb")
        if "dtrans" in ab:
            nc.gpsimd.memset(o_sb, 0.0)
        for g0 in ([] if "dtrans" in ab else range(0, NH, 4)):
            gn = min(4, NH - g0)
            tp = psT.tile([P, gn * P], p_dtype, tag="t4", name="tp")
            for j in range(gn):
                nc.tensor.transpose(
                    tp[:, j * P : (j + 1) * P],
                    oT_sb[:, (g0 + j) * P : (g0 + j + 1) * P],
                    ident_p,
                )
            for j in range(gn):
                nc.vector.tensor_scalar_mul(
                    out=o_sb[:, g0 + j, :],
                    in0=tp[:, j * P : (j + 1) * P],
                    scalar1=rl[:, qoff + g0 + j : qoff + g0 + j + 1],
                )
        nc.sync.dma_start(
            out=o_d[h].rearrange("(c p) d -> p c d", p=P)[:, qoff : qoff + NH, :],
            in_=o_sb,
        )

    # ---- main pipelined stream over all chunks of all heads ---------
    total = n_heads * NQ
    pend = None
    prep(0)
    sts_pend = emit_qk(0, 0)
    for g in range(total):
        h, qi = divmod(g, NQ)
        half, qq = divmod(qi, NH)
        qoff = half * NH
        hd = heads[h]
        if qq == 0:
            hd["pT"] = ptb.tile([P, NT, half_s], p_dtype, tag="pT", name=f"pT{g}")
        pT = hd["pT"]

        # issue next chunk's QK matmuls ahead of this chunk's transposes
        # so the PE never waits out the exp latency chain
        sts = sts_pend
        if g + 1 < total:
            hn, qin = divmod(g + 1, NQ)
            if qin == 0:
                prep(hn)
            sts_pend = emit_qk(hn, qin)

        p_row = prow.tile([P, seq], p_dtype, tag="prow", name=f"prow{g}")
        l_parts = small.tile([P, NTILE], F32, tag="lparts", name=f"lp{g}")
        if "exp" not in ab:
            for j, stt in enumerate(sts):
                nc.scalar.activation(
                    out=p_row[:, j * TW : (j + 1) * TW],
                    in_=stt,
                    func=EXP,
                    bias=hd["negm"],
                    accum_out=l_parts[:, j : j + 1],
                )
        if "lsum" not in ab:
            nc.vector.reduce_sum(hd["rlraw"][:, qi : qi + 1], l_parts, axis=AX)

        if "ptrans" not in ab:
            GRP = 8 if (p_dtype != F32 and NT % 8 == 0) else 4
            for gi, g0 in enumerate(range(0, NT, GRP)):
                gn = min(GRP, NT - g0)
                tp = psT.tile([P, gn * P], p_dtype, tag="t4", name="tp")
                for j in range(gn):
                    nc.tensor.transpose(
                        tp[:, j * P : (j + 1) * P],
                        p_row[:, (g0 + j) * P : (g0 + j + 1) * P],
                        ident_p,
                    )
                if "pcopy" not in ab:
                    dst = pT[:, g0 : g0 + gn, qq * P : (qq + 1) * P]
                    srcv = tp.rearrange("p (a b) -> p a b", a=gn)
                    nc.vector.tensor_copy(out=dst, in_=srcv)

        # interleave the pending half's PV matmuls across these chunks
        if pend is not None:
            per = NT // NH
            emit_pv(pend, range(per * qq, per * (qq + 1)))
            if qq == NH - 1:
                emit_phase_d(pend)
                pend = None

        if qq == NH - 1:
            if "lsum" not in ab:
                nc.vector.reciprocal(
                    hd["rl"][:, qoff : qoff + NH],
                    hd["rlraw"][:, qoff : qoff + NH],
                )
            osegs = []
            for c in range(0, half_s, SEG):
                ot = psO.tile([P, SEG], F32, tag="ot", name=f"ot{g}_{c}")
                osegs.append((ot, c, c + SEG))
            pend = dict(
                h=h, qoff=qoff, pT=pT, v_mm=hd["v_mm"], rl=hd["rl"],
                osegs=osegs,
            )

    if pend is not None:
        emit_pv(pend, range(NT))
        emit_phase_d(pend)


_NC_CACHE = {}


def _get_nc():
    key = (HEADS_PER_CORE, S, P_DTYPE, QK_MODE, ROWMAX_SUB)
    if key not in _NC_CACHE:
        _NC_CACHE[key] = build_attention_nc()
    return _NC_CACHE[key]


def kernel(query, key, value, scale_factor):
    global LAST_EXEC_NS
    from concourse.bass_utils import run_bass_kernel_spmd

    q = np.ascontiguousarray(np.asarray(query, dtype=np.float32).reshape(B * H, S, D))
    k = np.ascontiguousarray(np.asarray(key, dtype=np.float32).reshape(B * H, S, D))
    v = np.ascontiguousarray(np.asarray(value, dtype=np.float32).reshape(B * H, S, D))
    sc = np.ascontiguousarray(
        np.asarray(scale_factor, dtype=np.float32).reshape(B * H, 1)
    )

    nc = _get_nc()
    in_maps = []
    for c in range(N_CORES):
        sl = slice(c * HEADS_PER_CORE, (c + 1) * HEADS_PER_CORE)
        in_maps.append({"q": q[sl], "k": k[sl], "v": v[sl], "scale": sc[sl]})

    res = run_bass_kernel_spmd(nc, in_maps, list(range(N_CORES)), trace=TRACE)
    LAST_EXEC_NS = res.exec_time_ns
    outs = [np.asarray(res.results[c]["out"]) for c in range(N_CORES)]
    return np.concatenate(outs, axis=0).reshape(B, H, S, D).astype(np.float32)

